# revision 13
# baseline (speedup 1.0000x reference)
"""Trainium2 Bass kernel: GQA attention block (S=2048, HID=4096, 32 q heads /
8 kv heads, head dim 128, RoPE, causal), tensor-parallel over heads on 8
NeuronCores.

Sharding: core c owns q heads [4c..4c+4) and kv head c. wq/wk/wv are sharded
on their output dim, wo on its input dim; each core computes a partial
y_c = o_c @ wo_c.T and the host sums the 8 partials (the "all-reduce").

Everything on-device runs in a transposed [feature, seq] layout so that every
matmul streams wide moving operands:
  qT = wqT-blocks.T @ xT-blocks        (accumulated over K in PSUM)
  scoresT[kk, s] = kT-block.T @ qT     (per 128-key block, 512-seq chunk)
  probsT = exp(scoresT * 1/sqrt(128)), causal via narrowed matmuls + one
           triangular affine_select per diagonal block
  oT += vnat-block.T @ probsT          (accumulated over key blocks)
  denom = onesT @ probsT               (PE row-sum; softmax normalizer)
  yT = woT-blocks.T @ (oT * 1/denom)

RoPE uses a head-dim permutation (even dims first, odd dims second, folded
into the wq/wk rows on the host) so rotation pairs are the two partition
halves; the cross-half operands come from two partition-shifting ACT copies.

Matmuls run in float32r (full-rate fp32 PE mode) when USE_F32R is set.
"""

import os
import sys

import numpy as np

for _p in (
    "/root/.axon_site",
    "/root/.axon_site/_ro/trn_rl_repo",
    "/root/.axon_site/_ro/pypackages",
    "/opt/trn_rl_repo",
):
    if os.path.isdir(_p) and _p not in sys.path:
        sys.path.append(_p)

import concourse.bass as bass  # noqa: E402
import concourse.bacc as bacc  # noqa: E402
import concourse.mybir as mybir  # noqa: E402
from concourse import bass_utils  # noqa: E402
from concourse.tile import TileContext  # noqa: E402

F32 = mybir.dt.float32
F32R = mybir.dt.float32r

N_CORES = 8
SEQ = 2048
HID = 4096
NQ = 32
NKV = 8
HD = 128
THETA = 500000.0

HQ = NQ // N_CORES  # 4 q heads per core
QC = HQ * HD  # 512: per-core q feature slice
KB = SEQ // 128  # 16 key blocks
NKBLK = HID // 128  # 32 contraction blocks for the projections
NCHUNK = SEQ // 512  # 4 sequence chunks of 512
SCALE = 1.0 / float(np.sqrt(HD))

USE_F32R = True  # fp32r matmuls: ~4x faster PE, slightly reduced precision


def _build_body(tc, sb, sbw, ps, mdt):
    nc = tc.nc

    xT = nc.dram_tensor("xT", (HID, SEQ), mdt, kind="ExternalInput").ap()
    wq_sb_d = nc.dram_tensor("wq_sb", (128, NKBLK * QC), mdt, kind="ExternalInput").ap()
    wk_sb_d = nc.dram_tensor("wk_sb", (128, NKBLK * HD), mdt, kind="ExternalInput").ap()
    wv_sb_d = nc.dram_tensor("wv_sb", (128, NKBLK * HD), mdt, kind="ExternalInput").ap()
    wo_sb_d = nc.dram_tensor("wo_sb", (128, 4 * HID), mdt, kind="ExternalInput").ap()
    ones_d = nc.dram_tensor("ones_in", (128, 128), mdt, kind="ExternalInput").ap()
    cc_d = nc.dram_tensor("cc", (HD, SEQ), F32, kind="ExternalInput").ap()
    ss_d = nc.dram_tensor("ss", (HD, SEQ), F32, kind="ExternalInput").ap()
    yT_d = nc.dram_tensor("yT", (HID, SEQ), F32, kind="ExternalOutput").ap()

    # --- persistent SBUF tiles ---
    ones = sb.tile([128, 128], mdt, name="ones")
    nc.sync.dma_start(ones[:], ones_d[:])

    wq_t = sb.tile([128, NKBLK * QC], mdt, name="wq_t")
    wk_t = sb.tile([128, NKBLK * HD], mdt, name="wk_t")
    wv_t = sb.tile([128, NKBLK * HD], mdt, name="wv_t")
    for i in range(8):
        w = NKBLK * QC // 8
        nc.sync.dma_start(wq_t[:, i * w : (i + 1) * w], wq_sb_d[:, i * w : (i + 1) * w])
    for i in range(2):
        w = NKBLK * HD // 2
        nc.sync.dma_start(wk_t[:, i * w : (i + 1) * w], wk_sb_d[:, i * w : (i + 1) * w])
        nc.sync.dma_start(wv_t[:, i * w : (i + 1) * w], wv_sb_d[:, i * w : (i + 1) * w])

    qT = [sb.tile([128, SEQ], mdt, name=f"qT{h}") for h in range(HQ)]
    kT = sb.tile([128, SEQ], mdt, name="kT")
    vnat = sb.tile([128, KB * 128], mdt, name="vnat")

    # =================== phase 1: QKV projections + RoPE ===================
    def rope_inplace(dst, psrc, cct, sst, s0):
        """dst[:, s0:s0+512] = rope(psrc); partition rows 0:64 hold the even
        rope dims, 64:128 the odd ones (host permuted the weight rows)."""
        sw = sbw.tile([128, 512], F32, tag="ropetmp", bufs=6, name="sw")
        nc.scalar.copy(sw[0:64, :], psrc[64:128, :])
        nc.scalar.copy(sw[64:128, :], psrc[0:64, :])
        m1 = sbw.tile([128, 512], F32, tag="ropetmp", bufs=6, name="m1")
        m2 = sbw.tile([128, 512], F32, tag="ropetmp", bufs=6, name="m2")
        nc.vector.tensor_mul(m1[:], psrc[:], cct[:])
        nc.vector.tensor_mul(m2[:], sw[:], sst[:])
        nc.vector.tensor_sub(dst[0:64, s0 : s0 + 512], m1[0:64, :], m2[0:64, :])
        nc.vector.tensor_add(dst[64:128, s0 : s0 + 512], m1[64:128, :], m2[64:128, :])

    for sc_i in range(NCHUNK):
        s0 = sc_i * 512
        q_ps = [ps.tile([128, 512], F32, tag="acc", bufs=6, name=f"q_ps{h}") for h in range(HQ)]
        k_ps = ps.tile([128, 512], F32, tag="acc", bufs=6, name="k_ps")
        v_ps = ps.tile([128, 512], F32, tag="acc", bufs=6, name="v_ps")
        for k in range(NKBLK):
            xt = sbw.tile([128, 512], mdt, tag="stream", bufs=5, name="xt")
            nc.sync.dma_start(xt[:], xT[k * 128 : (k + 1) * 128, s0 : s0 + 512])
            st = k == 0
            sp = k == NKBLK - 1
            for h in range(HQ):
                wsl = wq_t[:, k * QC + h * 128 : k * QC + (h + 1) * 128]
                nc.tensor.matmul(q_ps[h][:], wsl, xt[:], start=st, stop=sp)
            nc.tensor.matmul(k_ps[:], wk_t[:, k * HD : (k + 1) * HD], xt[:], start=st, stop=sp)
            nc.tensor.matmul(v_ps[:], wv_t[:, k * HD : (k + 1) * HD], xt[:], start=st, stop=sp)
        cct = sbw.tile([128, 512], F32, tag="tbl", bufs=2, name="cct")
        sst = sbw.tile([128, 512], F32, tag="tbl", bufs=2, name="sst")
        nc.sync.dma_start(cct[:], cc_d[:, s0 : s0 + 512])
        nc.sync.dma_start(sst[:], ss_d[:, s0 : s0 + 512])
        for h in range(HQ):
            rope_inplace(qT[h], q_ps[h], cct, sst, s0)
        rope_inplace(kT, k_ps, cct, sst, s0)
        # v: PSUM holds vT chunk [d, s]; transpose 128-blocks into vnat [kk, d]
        vtmp = sbw.tile([128, 512], F32, tag="ropetmp", bufs=6, name="vtmp")
        nc.scalar.copy(vtmp[:], v_ps[:])
        for i in range(4):
            j = 4 * sc_i + i
            tp = ps.tile([128, 128], F32, tag="rot", bufs=2, name="tp")
            nc.tensor.transpose(tp[:], vtmp[:, i * 128 : (i + 1) * 128], ident_for(tc, sb))
            nc.scalar.copy(vnat[:, j * 128 : (j + 1) * 128], tp[:])

    # =================== phase 2: attention ===================
    # PE matmul dst partition offset must be 0, so each denominator gets its
    # own PSUM tile; heads run in pairs to stay within the 8 PSUM banks.
    for sc_i in range(NCHUNK):
        s0 = sc_i * 512
        jmax = 4 * sc_i + 3
        for hp in range(HQ // 2):
            heads = (2 * hp, 2 * hp + 1)
            o_ps = {h: ps.tile([128, 512], F32, tag="acc", bufs=6, name=f"o_ps{h}") for h in heads}
            den_ps = {h: ps.tile([128, 512], F32, tag="acc", bufs=6, name=f"den_ps{h}") for h in heads}
            for j in range(jmax + 1):
                # causal: columns below s0+off are fully masked for this block
                off = 128 * max(0, j - 4 * sc_i)
                for h in heads:
                    s_ps = ps.tile([128, 512], F32, tag="rot", bufs=2, name="s_ps")
                    nc.tensor.matmul(
                        s_ps[:, off:512],
                        kT[:, j * 128 : (j + 1) * 128],
                        qT[h][:, s0 + off : s0 + 512],
                        start=True,
                        stop=True,
                    )
                    et = sbw.tile([128, 512], mdt, tag="stream", bufs=5, name="et")
                    nc.scalar.activation(
                        et[:, off:512], s_ps[:, off:512],
                        mybir.ActivationFunctionType.Exp, scale=SCALE,
                    )
                    g = j - 4 * sc_i
                    if g >= 0:  # diagonal block: keep keys kk <= s in block
                        nc.gpsimd.affine_select(
                            out=et[:, g * 128 : (g + 1) * 128],
                            in_=et[:, g * 128 : (g + 1) * 128],
                            compare_op=mybir.AluOpType.is_ge,
                            fill=0.0,
                            base=0,
                            pattern=[[1, 128]],
                            channel_multiplier=-1,
                        )
                    st = j == 0
                    sp = j == jmax
                    nc.tensor.matmul(
                        o_ps[h][:, off:512], vnat[:, j * 128 : (j + 1) * 128],
                        et[:, off:512], start=st, stop=sp,
                    )
                    nc.tensor.matmul(
                        den_ps[h][0:1, off:512], ones[:, 0:1],
                        et[:, off:512], start=st, stop=sp,
                    )
            # normalize: oT = o_ps / denom, written over the dead qT chunk.
            # Broadcast denom across partitions with a K=1 PE matmul
            # (ones-column @ den-row), then reciprocal + multiply on DVE.
            for h in heads:
                den_sb = sbw.tile([1, 512], mdt, tag="den", bufs=4, name="den_sb")
                nc.scalar.copy(den_sb[0:1, :], den_ps[h][0:1, :])
                bc_ps = ps.tile([128, 512], F32, tag="rot", bufs=2, name="bc_ps")
                nc.tensor.matmul(bc_ps[:], ones[0:1, :], den_sb[0:1, :], start=True, stop=True)
                rec_bc = sbw.tile([128, 512], F32, tag="bcast", bufs=2, name="rec_bc")
                nc.vector.reciprocal(rec_bc[:], bc_ps[:])
                nc.vector.tensor_mul(qT[h][:, s0 : s0 + 512], o_ps[h][:], rec_bc[:])

    oT = qT  # qT tiles now hold the normalized attention output

    # =================== phase 3: output projection ===================
    wo_r = wo_sb_d.rearrange("p (c j) -> p c j", c=4)
    neng = 0
    for jb in range(HID // 128):
        wob = sbw.tile([128, 512], mdt, tag="stream", bufs=5, name="wob")
        nc.sync.dma_start(
            wob[:].rearrange("p (c j) -> p c j", c=4),
            wo_r[:, :, jb * 128 : (jb + 1) * 128],
        )
        for sc_i in range(NCHUNK):
            s0 = sc_i * 512
            y_ps = ps.tile([128, 512], F32, tag="rot", bufs=2, name="y_ps")
            for cb in range(4):
                nc.tensor.matmul(
                    y_ps[:],
                    wob[:, cb * 128 : (cb + 1) * 128],
                    oT[cb][:, s0 : s0 + 512],
                    start=(cb == 0),
                    stop=(cb == 3),
                )
            yst = sbw.tile([128, 512], F32, tag="ropetmp", bufs=6, name="yst")
            if neng % 2 == 0:
                nc.scalar.copy(yst[:], y_ps[:])
            else:
                nc.vector.tensor_copy(yst[:], y_ps[:])
            neng += 1
            nc.sync.dma_start(yT_d[jb * 128 : (jb + 1) * 128, s0 : s0 + 512], yst[:])


_IDENT = {}


def ident_for(tc, sb):
    if "t" not in _IDENT:
        from concourse.masks import make_identity

        ident = sb.tile([128, 128], F32, name="ident")
        make_identity(tc.nc, ident)
        _IDENT["t"] = ident
    return _IDENT["t"]


_NC_CACHE = {}


def _get_nc():
    key = ("v2", USE_F32R)
    if key not in _NC_CACHE:
        _IDENT.clear()
        mdt = F32R if USE_F32R else F32
        nc = bacc.Bacc("TRN2", target_bir_lowering=False, debug=False, num_devices=N_CORES)
        with TileContext(nc) as tc:
            with (
                tc.tile_pool(name="sb", bufs=1) as sb,
                tc.tile_pool(name="sbw", bufs=1) as sbw,
                tc.tile_pool(name="ps", bufs=1, space="PSUM") as ps,
            ):
                _build_body(tc, sb, sbw, ps, mdt)
        nc.compile()
        _NC_CACHE[key] = nc
    return _NC_CACHE[key]


_ROPE_PERM = np.concatenate([np.arange(0, 128, 2), np.arange(1, 128, 2)])


def _rope_tables(start_pos):
    freqs = 1.0 / (THETA ** (np.arange(0, HD, 2, dtype=np.float64) / HD))
    t = np.arange(start_pos, start_pos + SEQ, dtype=np.float64)
    ang = np.outer(t, freqs)  # [SEQ, 64]
    cosT = np.cos(ang).T.astype(np.float32)  # [64, SEQ]
    sinT = np.sin(ang).T.astype(np.float32)
    cc = np.ascontiguousarray(np.concatenate([cosT, cosT], axis=0))
    ss = np.ascontiguousarray(np.concatenate([sinT, sinT], axis=0))
    return cc, ss


def _to_kblock_layout(wT, cwidth):
    """[HID, cwidth] feature-major weight -> [128, NKBLK*cwidth] with k-block
    k at columns [k*cwidth, (k+1)*cwidth)."""
    return np.ascontiguousarray(
        wT.reshape(NKBLK, 128, cwidth).transpose(1, 0, 2).reshape(128, NKBLK * cwidth)
    )


def make_in_maps(x, wq, wk, wv, wo, start_pos):
    x = np.asarray(x, dtype=np.float32)
    wq = np.asarray(wq, dtype=np.float32)
    wk = np.asarray(wk, dtype=np.float32)
    wv = np.asarray(wv, dtype=np.float32)
    wo = np.asarray(wo, dtype=np.float32)
    sp = int(start_pos)

    xT = np.ascontiguousarray(x.T)
    cc, ss = _rope_tables(sp)
    woT = np.ascontiguousarray(wo.T)  # [in=c, out=j]
    ones_in = np.ones((128, 128), dtype=np.float32)

    in_maps = []
    for c in range(N_CORES):
        wq_c = wq[c * QC : (c + 1) * QC, :]  # [512, HID]
        wq_c = wq_c.reshape(HQ, HD, HID)[:, _ROPE_PERM, :].reshape(QC, HID)
        wk_c = wk[c * HD : (c + 1) * HD, :][_ROPE_PERM, :]  # [128, HID]
        wv_c = wv[c * HD : (c + 1) * HD, :]  # [128, HID]
        wq_sbm = _to_kblock_layout(np.ascontiguousarray(wq_c.T), QC)
        wk_sbm = _to_kblock_layout(np.ascontiguousarray(wk_c.T), HD)
        wv_sbm = _to_kblock_layout(np.ascontiguousarray(wv_c.T), HD)
        woT_c = woT[c * QC : (c + 1) * QC, :]  # [512, HID]
        wo_sbm = np.ascontiguousarray(
            woT_c.reshape(4, 128, HID).transpose(1, 0, 2).reshape(128, 4 * HID)
        )
        in_maps.append(
            {
                "xT": xT,
                "wq_sb": wq_sbm,
                "wk_sb": wk_sbm,
                "wv_sb": wv_sbm,
                "wo_sb": wo_sbm,
                "ones_in": ones_in,
                "cc": cc,
                "ss": ss,
            }
        )
    return in_maps


def _assemble(results):
    acc = results[0]["yT"].astype(np.float32)
    for r in results[1:]:
        acc = acc + r["yT"]
    return np.ascontiguousarray(acc.T)


def kernel(x, wq, wk, wv, wo, start_pos):
    nc = _get_nc()
    in_maps = make_in_maps(x, wq, wk, wv, wo, start_pos)
    res = bass_utils.run_bass_kernel_spmd(nc, in_maps, core_ids=list(range(N_CORES)))
    return _assemble(res.results)


def run_traced(x, wq, wk, wv, wo, start_pos):
    """Like kernel() but with NTFF tracing; returns (out, BassKernelResults)."""
    nc = _get_nc()
    in_maps = make_in_maps(x, wq, wk, wv, wo, start_pos)
    res = bass_utils.run_bass_kernel_spmd(
        nc, in_maps, core_ids=list(range(N_CORES)), trace=True
    )
    return _assemble(res.results), res
